# revision 12
# baseline (speedup 1.0000x reference)
"""ArcDecoder distributed Bass kernel for 8 TRN2 NeuronCores.

Problem: for each arc e with endpoints (s, d):
    h   = concat(z[s], z[d])                # [256]
    h1  = relu(W1 @ h + b1)                 # [128]
    out = W2 @ h1 + b2                      # scalar

Math transform: W1 @ concat(z_s, z_d) = W1a @ z_s + W1b @ z_d, so per-node
tables are precomputed once:
    A~[n] = (z[n] @ W1a.T) * |W2|,  B~[n] = (z[n] @ W1b.T) * |W2| + |W2|*b1
and out[e] = sum_j sign(W2_j) * relu(A~[s,j] + B~[d,j]) + b2.

Design (v2 - SBUF-resident tables):
  * 2D arc sharding over the 8 cores: 4 src-ranges x 2 dst-ranges of the
    node id space.  Core (i,j) handles arcs with src in range i (25000
    nodes) and dst in range j (50000 nodes).
  * Each core computes its A~ rows (25088 slots) and B~ rows (50176 slots)
    with matmuls and keeps them in SBUF as 6 gather sub-tables of 12544
    tokens each ([128 part, 98 ranks, 256B rows]; token t -> partition
    t%128, rank t//128) -- the natural matmul tile layout, no transpose,
    no DRAM round trip.
  * Arcs are host-bucketed into 8 (a_sub, b_sub) groups; per 2048-arc chunk
    an SBUF-source transposed dma_gather pulls A~[src]/B~[dst] columns into
    [128 j, 2048 arc] tiles; DVE adds, DVE relu; the 128-way j-reduction
    with the sign weights runs on the Tensor engine via basis matmuls that
    accumulate 32 x [1, 512] row-sums into a [32, 512] PSUM tile per group.
  * Group capacity 16384 (mean occupancy 15625), padding gathers token 0.

Sharding: host buckets arcs by (src range, dst range); z replicated; no
collectives.
"""

import numpy as np

# ---------------- problem constants (hardcoded, per the task spec) ----------
N_NODES = 100000
HIDDEN = 128
N_ARCS = 1000000
N_CORES = 8

P = 128  # SBUF partitions

# ---------------- sharding / tiling configuration ---------------------------
SRC_RANGES = 4            # src id space split (range size 25000 real nodes)
DST_RANGES = 2            # dst id space split (range size 50000 real nodes)
SRC_RANGE = N_NODES // SRC_RANGES   # 25000
DST_RANGE = N_NODES // DST_RANGES   # 50000

SUB_TOK = 12544           # tokens per gather sub-table (98 ranks x 128)
SUB_RANKS = SUB_TOK // P  # 98
A_SUBS = 2                # sub-tables covering a src range  (2*12544 >= 25000)
B_SUBS = 4                # sub-tables covering a dst range  (4*12544 >= 50000)
NGRP = A_SUBS * B_SUBS    # 8 arc groups per core

CAP = 16384               # arcs per group (mean 15625, +6 sigma)
CHUNK = 2048              # arcs per gather chunk
CHUNKS_PER_GRP = CAP // CHUNK      # 8
SLICE = 512               # arcs per reduce matmul (PSUM bank free width)
SLICES_PER_GRP = CAP // SLICE      # 32
E_OUT = NGRP * CAP        # 131072 device outputs per core

ZCHUNK_TILES = 16         # node tiles per z-chunk DMA in phase 1

# None -> spread gathers over the 4 SWDGE queues (queue per scheduled DMASW
# lane, fixed up post-compile); an int pins every gather to that queue.
GATHER_QUEUES_FIXED = 0

# group build/processing order: (a_sub, b_sub)
GRP_ORDER = [(0, 0), (1, 0), (0, 1), (1, 1), (0, 2), (1, 2), (0, 3), (1, 3)]


def _build_graph(b1_nonzero=False):
    """Build the SPMD single-core graph (all 8 cores run this same graph)."""
    import concourse.bass as bass
    from concourse import bacc, mybir, tile

    BF16 = mybir.dt.bfloat16
    F32 = mybir.dt.float32
    I16 = mybir.dt.int16
    H = HIDDEN

    nc = bacc.Bacc(None, target_bir_lowering=False, num_swdge_queues=4)
    with tile.TileContext(nc) as tc:
        with tc.tile_pool(name="dram", bufs=1, space="DRAM") as dram:
            # z transposed [hidden, node], bf16, zero-padded past N_NODES
            z_T = dram.tile([P, 100352], BF16, kind="ExternalInput",
                            name="z_T", uniquify=False)
            # phase-1 matmul rhs: [h, j] halves scaled by |W2|
            wsa = dram.tile([P, H], BF16, kind="ExternalInput",
                            name="wsa", uniquify=False)
            wsb = dram.tile([P, H], BF16, kind="ExternalInput",
                            name="wsb", uniquify=False)
            # beta = |W2|*b1 broadcast [P, H] (only used when b1 != 0)
            beta = dram.tile([P, H], F32, kind="ExternalInput",
                             name="beta", uniquify=False)
            # basis column source: sgn vector (bf16) [P, 1]
            sgnv = dram.tile([P, 1], BF16, kind="ExternalInput",
                             name="sgnv", uniquify=False)
            b2r = dram.tile([P, 1], F32, kind="ExternalInput",
                            name="b2r", uniquify=False)
            # wrapped int16 gather indices, group-major [P, NGRP*CAP/16]
            isrc = dram.tile([P, NGRP * CAP // 16], I16, kind="ExternalInput",
                             name="isrc", uniquify=False)
            idst = dram.tile([P, NGRP * CAP // 16], I16, kind="ExternalInput",
                             name="idst", uniquify=False)
            outv = dram.tile([E_OUT], F32, kind="ExternalOutput",
                             name="outv", uniquify=False)

            with tc.tile_pool(name="consts", bufs=1) as cpool:
                wsa_s = cpool.tile([P, H], BF16, name="wsa_s")
                nc.sync.dma_start(out=wsa_s[:], in_=wsa[:])
                wsb_s = cpool.tile([P, H], BF16, name="wsb_s")
                nc.sync.dma_start(out=wsb_s[:], in_=wsb[:])
                b2_s = cpool.tile([P, 1], F32, name="b2_s")
                nc.sync.dma_start(out=b2_s[:], in_=b2r[:])
                if b1_nonzero:
                    beta_s = cpool.tile([P, H], F32, name="beta_s")
                    nc.sync.dma_start(out=beta_s[:], in_=beta[:])
                # basis buffer: [P, 2*P] bf16, zeros except column P = sgn
                basis = cpool.tile([P, 2 * P], BF16, name="basis")
                nc.vector.memset(basis[:], 0.0)
                sgn_s = cpool.tile([P, 1], BF16, name="sgn_s")
                nc.sync.dma_start(out=sgn_s[:], in_=sgnv[:])
                nc.scalar.copy(basis[:, P:P + 1], sgn_s[:])

                # persistent SBUF gather tables: 6 sub-tables
                # layout [P, SUB_RANKS, H]: token t -> (partition t%128,
                # rank t//128), row = 256B of bf16
                tabs = {}
                for name in ("A0", "A1", "B0", "B1", "B2", "B3"):
                    tabs[name] = cpool.tile([P, SUB_RANKS, H], BF16,
                                            name=f"tab{name}")

                with tc.tile_pool(name="zc", bufs=2) as zpool, \
                     tc.tile_pool(name="ps1", bufs=4, space="PSUM") as ps1, \
                     tc.tile_pool(name="gx", bufs=2) as gxpool, \
                     tc.tile_pool(name="gy", bufs=2) as gypool, \
                     tc.tile_pool(name="rb", bufs=2) as rbpool, \
                     tc.tile_pool(name="gi", bufs=2) as gipool, \
                     tc.tile_pool(name="gj", bufs=2) as gjpool, \
                     tc.tile_pool(name="ps2", bufs=2, space="PSUM") as ps2, \
                     tc.tile_pool(name="ob", bufs=2) as obpool:

                    # ---- phase-1 builder: fill one sub-table ----
                    def build_sub(tab, z_col0, rhs_s, is_b):
                        """tab: table tile; z_col0: first z_T column;
                        rhs_s: wsa_s or wsb_s."""
                        for c0 in range(0, SUB_RANKS, ZCHUNK_TILES):
                            ct = min(ZCHUNK_TILES, SUB_RANKS - c0)
                            zc = zpool.tile([P, ZCHUNK_TILES * P], BF16,
                                            tag="zc")
                            nc.sync.dma_start(
                                out=zc[:, :ct * P],
                                in_=z_T[:, z_col0 + c0 * P:
                                        z_col0 + (c0 + ct) * P])
                            for t in range(ct):
                                ps = ps1.tile([P, H], F32, tag="ps")
                                nc.tensor.matmul(
                                    ps[:], lhsT=zc[:, t * P:(t + 1) * P],
                                    rhs=rhs_s[:], start=True, stop=True)
                                # PSUM f32 -> bf16 table row (ACT engine);
                                # +beta via DVE when b1 != 0
                                if is_b and b1_nonzero:
                                    nc.vector.scalar_tensor_tensor(
                                        out=tab[:, c0 + t, :], in0=ps[:],
                                        scalar=1.0, in1=beta_s[:],
                                        op0=mybir.AluOpType.mult,
                                        op1=mybir.AluOpType.add)
                                else:
                                    nc.scalar.copy(tab[:, c0 + t, :], ps[:])

                    # ---- phase-2: process one group ----
                    # SWDGE queue: the tile framework binds each Pool-DMA
                    # to DMASW lane (emission_idx % 8); a lane's semaphore
                    # is locked to one queue, so queue = emission_idx % 4.
                    gather_ctr = [0]

                    def do_group(gi_idx, a_sub, b_sub):
                        atab = tabs[f"A{a_sub}"]
                        btab = tabs[f"B{b_sub}"]
                        i0 = gi_idx * (CAP // 16)  # idx col offset
                        ia = gipool.tile([P, CAP // 16], I16, tag="ia")
                        nc.sync.dma_start(out=ia[:],
                                          in_=isrc[:, i0:i0 + CAP // 16])
                        ib = gjpool.tile([P, CAP // 16], I16, tag="ib")
                        nc.sync.dma_start(out=ib[:],
                                          in_=idst[:, i0:i0 + CAP // 16])
                        psb = ps2.tile([SLICES_PER_GRP, SLICE], F32,
                                       tag="psb")
                        for c in range(CHUNKS_PER_GRP):
                            gA = gxpool.tile([P, 1, CHUNK], BF16, tag="gA")
                            nc.gpsimd.dma_gather(
                                gA[:], atab[:], ia[:, c * (CHUNK // 16):
                                                   (c + 1) * (CHUNK // 16)],
                                CHUNK, CHUNK, H,
                                transpose=True,
                                queue_num=GATHER_QUEUES_FIXED if
                                GATHER_QUEUES_FIXED is not None else
                                gather_ctr[0] % 4,
                                single_packet=False,
                                sbuf_tokens_per_rank=P,
                                sbuf_free_dim_per_rank=2 * H,
                                sbuf_byte_offset=0)
                            gather_ctr[0] += 1
                            gB = gypool.tile([P, 1, CHUNK], BF16, tag="gB")
                            nc.gpsimd.dma_gather(
                                gB[:], btab[:], ib[:, c * (CHUNK // 16):
                                                   (c + 1) * (CHUNK // 16)],
                                CHUNK, CHUNK, H,
                                transpose=True,
                                queue_num=GATHER_QUEUES_FIXED if
                                GATHER_QUEUES_FIXED is not None else
                                gather_ctr[0] % 4,
                                single_packet=False,
                                sbuf_tokens_per_rank=P,
                                sbuf_free_dim_per_rank=2 * H,
                                sbuf_byte_offset=0)
                            gather_ctr[0] += 1
                            # sum + relu on DVE (bf16, 2 elem/cy)
                            rb = rbpool.tile([P, CHUNK], BF16, tag="rb")
                            nc.vector.tensor_tensor(
                                out=rb[:], in0=gA[:, 0, :], in1=gB[:, 0, :],
                                op=mybir.AluOpType.add)
                            nc.vector.tensor_scalar_max(
                                out=rb[:], in0=rb[:], scalar1=0.0)
                            # sign-weighted j-reduction: basis matmuls
                            for s in range(CHUNK // SLICE):
                                k = c * (CHUNK // SLICE) + s
                                nc.tensor.matmul(
                                    psb[:],
                                    lhsT=basis[:, P - k:2 * P - k
                                               ][:, :SLICES_PER_GRP],
                                    rhs=rb[:, s * SLICE:(s + 1) * SLICE],
                                    start=(k == 0),
                                    stop=(k == SLICES_PER_GRP - 1))
                        ob = obpool.tile([SLICES_PER_GRP, SLICE], F32,
                                         tag="ob")
                        nc.vector.tensor_scalar_add(
                            out=ob[:], in0=psb[:],
                            scalar1=b2_s[:SLICES_PER_GRP, 0:1])
                        nc.sync.dma_start(
                            out=outv[gi_idx * CAP:(gi_idx + 1) * CAP
                                     ].rearrange("(s c) -> s c", c=SLICE),
                            in_=ob[:])

                    # ---- schedule: interleave builds and groups ----
                    # src range cols at 0 (local), dst range cols at offset
                    # handled by host via z layout: core reads its own
                    # ranges from the replicated z_T using host-provided
                    # per-core col offsets... cols are fixed per core, but
                    # the graph is SPMD-shared, so the host instead rolls
                    # z_T per core so that:
                    #   cols [0, 25088)        = this core's src range
                    #   cols [32768, 82944)    = this core's dst range
                    build_sub(tabs["A0"], 0, wsa_s, False)
                    build_sub(tabs["B0"], 32768, wsb_s, True)
                    do_group(0, *GRP_ORDER[0])           # (0,0)
                    build_sub(tabs["A1"], SUB_TOK, wsa_s, False)
                    do_group(1, *GRP_ORDER[1])           # (1,0)
                    build_sub(tabs["B1"], 32768 + SUB_TOK, wsb_s, True)
                    do_group(2, *GRP_ORDER[2])
                    do_group(3, *GRP_ORDER[3])
                    build_sub(tabs["B2"], 32768 + 2 * SUB_TOK, wsb_s, True)
                    do_group(4, *GRP_ORDER[4])
                    do_group(5, *GRP_ORDER[5])
                    build_sub(tabs["B3"], 32768 + 3 * SUB_TOK, wsb_s, True)
                    do_group(6, *GRP_ORDER[6])
                    do_group(7, *GRP_ORDER[7])
    nc.compile()
    if GATHER_QUEUES_FIXED is None:
        # The tile scheduler reorders instructions; SWDGE DMA sems (DMASW
        # lanes) are assigned in scheduled order and each lane's sem is
        # locked to one queue.  Rewrite queue_num from the assigned lane.
        from concourse.tile_sem_assignment import PROC_NAME_TO_IDX
        lane_base = PROC_NAME_TO_IDX["DMASW0"]
        for bb in nc.m.functions[0].blocks:
            for inst in bb.instructions:
                if type(inst).__name__ == "InstDMAGatherAnt":
                    inst.queue_num = (inst.bass_scheduled_proc - lane_base) % 4
    return nc


def _host_prep(z, pot_arcs, W1, b1, W2, b2, n_cores=N_CORES):
    """Stage inputs: dtype/layout conversion, arc bucketing, sharding.

    Returns (in_maps, slot, counts) where slot[i] is the device output
    position of arc i within its core's output vector.
    """
    import ml_dtypes

    bf16 = ml_dtypes.bfloat16
    H = HIDDEN
    z = np.asarray(z, np.float32)
    W1 = np.asarray(W1, np.float32)
    b1 = np.asarray(b1, np.float32).reshape(-1)
    W2 = np.asarray(W2, np.float32).reshape(-1)
    b2 = np.asarray(b2, np.float32).reshape(-1)
    arcs = np.asarray(pot_arcs)

    absw2 = np.abs(W2)
    sgn = np.sign(W2).astype(np.float32)
    wsa = np.ascontiguousarray((W1[:, :H] * absw2[:, None]).T).astype(bf16)
    wsb = np.ascontiguousarray((W1[:, H:] * absw2[:, None]).T).astype(bf16)
    beta = np.broadcast_to((absw2 * b1)[None, :], (P, H)).copy().astype(
        np.float32)
    sgnv = sgn[:, None].astype(bf16)
    b2r = np.full((P, 1), b2[0], np.float32)

    zT_full = np.zeros((P, 100352), bf16)
    zT_full[:, :z.shape[0]] = np.ascontiguousarray(z.T).astype(bf16)

    src = np.asarray(arcs[:, 0], np.int64)
    dst = np.asarray(arcs[:, 1], np.int64)
    ri = src // SRC_RANGE          # 0..3
    rj = dst // DST_RANGE          # 0..1
    core = ri * DST_RANGES + rj    # 0..7

    # group order position lookup: (a_sub, b_sub) -> gi
    grp_pos = np.zeros((A_SUBS, B_SUBS), np.int64)
    for gi, (a, b) in enumerate(GRP_ORDER):
        grp_pos[a, b] = gi

    in_maps = []
    slot_all = np.empty(arcs.shape[0], np.int64)
    core_of = np.empty(arcs.shape[0], np.int64)
    core_pos = np.empty(arcs.shape[0], np.int64)
    for c in range(n_cores):
        i, j = divmod(c, DST_RANGES)
        m = np.nonzero(core == c)[0]
        ls = src[m] - i * SRC_RANGE          # 0..24999
        ld = dst[m] - j * DST_RANGE          # 0..49999
        a_sub = ls // SUB_TOK                # 0..1
        b_sub = ld // SUB_TOK                # 0..3
        gi_arr = grp_pos[a_sub, b_sub]
        order = np.argsort(gi_arr, kind="stable")
        counts = np.bincount(gi_arr, minlength=NGRP)
        if counts.max() > CAP:
            raise RuntimeError(f"group overflow: {counts.max()} > {CAP}")
        starts = np.zeros(NGRP, np.int64)
        starts[1:] = np.cumsum(counts)[:-1]
        pos_sorted = np.arange(m.size) - starts[gi_arr[order]]
        slot_sorted = gi_arr[order] * CAP + pos_sorted
        core_of[m] = c
        core_pos_m = np.empty(m.size, np.int64)
        core_pos_m[order] = slot_sorted
        core_pos[m] = core_pos_m

        la = np.zeros(NGRP * CAP, np.int16)   # padding -> token 0 (valid)
        lb = np.zeros(NGRP * CAP, np.int16)
        la[slot_sorted] = (ls % SUB_TOK)[order].astype(np.int16)
        lb[slot_sorted] = (ld % SUB_TOK)[order].astype(np.int16)
        # wrapped idx layout: position i -> (partition i%16, free i//16),
        # replicated across the 8 Q7 core pairs (128 partitions)
        wa = np.ascontiguousarray(
            la.reshape(-1, 16).T.reshape(16, -1))
        wb = np.ascontiguousarray(
            lb.reshape(-1, 16).T.reshape(16, -1))

        # roll z_T so this core's ranges sit at fixed graph offsets:
        # src range -> cols [0, 25088), dst range -> cols [32768, 82944)
        zT = np.zeros((P, 100352), bf16)
        zT[:, 0:A_SUBS * SUB_TOK] = _slice_pad(
            zT_full, i * SRC_RANGE, A_SUBS * SUB_TOK)
        zT[:, 32768:32768 + B_SUBS * SUB_TOK] = _slice_pad(
            zT_full, j * DST_RANGE, B_SUBS * SUB_TOK)
        in_maps.append(dict(
            z_T=zT, wsa=wsa, wsb=wsb, beta=beta, sgnv=sgnv, b2r=b2r,
            isrc=np.tile(wa, (8, 1)), idst=np.tile(wb, (8, 1))))
    slot_all = core_pos
    return in_maps, core_of, slot_all


def _slice_pad(zT, col0, width):
    """zT[:, col0:col0+width], zero-padded past the array end."""
    out = np.zeros((zT.shape[0], width), zT.dtype)
    hi = min(col0 + width, zT.shape[1])
    if hi > col0:
        out[:, :hi - col0] = zT[:, col0:hi]
    return out


_GRAPH_CACHE = {}


def _get_graph(b1_nonzero):
    key = (CAP, CHUNK, b1_nonzero)
    if key not in _GRAPH_CACHE:
        _GRAPH_CACHE[key] = _build_graph(b1_nonzero=b1_nonzero)
    return _GRAPH_CACHE[key]


def kernel(z, pot_arcs, W1, b1, W2, b2):
    from concourse.bass_utils import run_bass_kernel_spmd

    nc = _get_graph(bool(np.any(np.asarray(b1, np.float32))))
    in_maps, core_of, slot = _host_prep(z, pot_arcs, W1, b1, W2, b2)
    res = run_bass_kernel_spmd(nc, in_maps, core_ids=list(range(N_CORES)))
    out = np.empty(N_ARCS, np.float32)
    for c in range(N_CORES):
        m = core_of == c
        dev = np.asarray(res.results[c]["outv"], np.float32)
        out[m] = dev[slot[m]]
    return out


# revision 23
# speedup vs baseline: 2.2620x; 2.2620x over previous
"""ArcDecoder distributed Bass kernel for 8 TRN2 NeuronCores.

Problem: for each arc e with endpoints (s, d):
    h   = concat(z[s], z[d])                # [256]
    h1  = relu(W1 @ h + b1)                 # [128]
    out = W2 @ h1 + b2                      # scalar

Math transform: W1 @ concat(z_s, z_d) = W1a @ z_s + W1b @ z_d, so per-node
tables are precomputed once (100k nodes instead of 1M arcs):
    A~[n] = (z[n] @ W1a.T) * |W2|,  B~[n] = (z[n] @ W1b.T) * |W2| + |W2|*b1
stored interleaved in bf16 as T[n] = [A~[n], B~[n]] (512B rows).  Then
    out[e] = sum_j sign(W2_j) * relu(A~[s,j] + B~[d,j]) + b2
i.e. per arc: two 256B gathers + an add + one fused max0/mul DVE op + a
segmented reduce.  No per-arc matmul.

Gather: `dma_gather` (the Q7 SWDGE gather) takes int16 indices, so nodes are
split into 4 ranges of 25024 rows and each core's arcs are host-bucketed into
16 (src_range, dst_range) groups; each group's gathers use the range base as
the table offset so all indices fit int16.  Groups are padded to a static
capacity with index-0 dummies; the padding is discarded on the host.

Sharding: arcs split evenly across the 8 cores; z/weights replicated.
No collectives.
"""

import numpy as np

# ---------------- problem constants (hardcoded, per the task spec) ----------
N_NODES = 100000
HIDDEN = 128
N_ARCS = 1000000
N_CORES = 8

P = 128  # SBUF partitions

# ---------------- tiling configuration --------------------------------------
NRANGE = 4
# range size tile-aligned so each range's table is whole 128-node tiles
RSIZE = ((N_NODES + NRANGE * P - 1) // (NRANGE * P)) * P  # 25088
NODE_PAD = NRANGE * RSIZE  # 100352
NGRP = NRANGE * NRANGE  # 16

E_PER_CORE = N_ARCS // N_CORES  # 125000
# group capacity: mean count is E_PER_CORE/16 = 7812.5, sigma ~86 for uniform
# random arcs; 65 tiles = 8320 is ~5.9 sigma above the mean.
CTILES = 65
CAP = CTILES * P  # 8320
E_OUT = NGRP * CAP  # 133120 device outputs per core

CHUNK_TILES = 16  # node tiles per z-chunk DMA in the precompute phase
WB_TILES = 16  # node tiles per T-writeback DMA (1 MB)

# knobs
ADD_VIA_CCE = False  # A += B via SWDGE SBUF->SBUF accumulate DMA (else DVE)
GATHER_QUEUES = 4  # SWDGE queues to spread dma_gathers over (1..4)


def _build_graph(node_pad, rsize, cap, chunk_tiles, b1_nonzero=False,
                 add_via_cce=ADD_VIA_CCE, gather_queues=GATHER_QUEUES):
    """Build the SPMD single-core graph (all 8 cores run this same graph)."""
    import concourse.bass as bass
    from concourse import bacc, mybir, tile
    from concourse.masks import make_identity

    BF16 = mybir.dt.bfloat16
    F32 = mybir.dt.float32
    I16 = mybir.dt.int16
    H = HIDDEN
    NT = node_pad // P
    nrange = node_pad // rsize
    ngrp = nrange * nrange
    ctiles = cap // P
    S = cap // 16  # idx free-dim per group (16-partition wrap)
    e_out = ngrp * cap
    rcols = ngrp * ctiles  # result columns (= e_out / 128)
    rcols_pad = ((rcols + P - 1) // P) * P

    nc = bacc.Bacc(None, target_bir_lowering=False,
                   num_swdge_queues=gather_queues)
    with tile.TileContext(nc) as tc:
        with tc.tile_pool(name="dram", bufs=1, space="DRAM") as dram:
            z_T = dram.tile([P, node_pad], BF16, kind="ExternalInput",
                            name="z_T", uniquify=False)
            wcat = dram.tile([P, 2 * H], BF16, kind="ExternalInput",
                             name="wcat", uniquify=False)
            beta = dram.tile([P, 2 * H], F32, kind="ExternalInput",
                             name="beta", uniquify=False)
            sgn = dram.tile([P, H], BF16, kind="ExternalInput",
                            name="sgn", uniquify=False)
            b2r = dram.tile([P, 1], F32, kind="ExternalInput",
                            name="b2r", uniquify=False)
            isrc = dram.tile([P, ngrp * S], I16, kind="ExternalInput",
                             name="isrc", uniquify=False)
            idst = dram.tile([P, ngrp * S], I16, kind="ExternalInput",
                             name="idst", uniquify=False)
            outv = dram.tile([e_out], F32, kind="ExternalOutput",
                             name="outv", uniquify=False)
            # partition-major table layout: node (local) n lives at
            # [p = n % 128, t = n // 128] so the phase-1 writeback is
            # contiguous 8KB per partition (128 descriptors per chunk
            # instead of 2048 x 512B row descriptors).  Gather row index
            # for node n is (n % 128) * RT + n // 128 (host-computed).
            RT = rsize // P
            Ttabs = [dram.tile([P, RT, 2 * H], BF16, kind="Internal",
                                name=f"Ttab{r}", uniquify=False)
                     for r in range(nrange)]

            with tc.tile_pool(name="consts", bufs=1) as cpool:
                wcat_s = cpool.tile([P, 2 * H], BF16, name="wcat_s")
                nc.sync.dma_start(out=wcat_s[:], in_=wcat[:])
                beta_s = cpool.tile([P, 2 * H], F32, name="beta_s")
                nc.sync.dma_start(out=beta_s[:], in_=beta[:])
                sgn_s = cpool.tile([P, H], BF16, name="sgn_s")
                nc.sync.dma_start(out=sgn_s[:], in_=sgn[:])
                b2_s = cpool.tile([P, 1], F32, name="b2_s")
                nc.sync.dma_start(out=b2_s[:], in_=b2r[:])

                # ---- Phase 1: per-node tables T = [A~ | B~] ----
                # all pools share one scope: phase-2 tiles must NOT alias
                # phase-1 SBUF (aliasing would serialize the phases and has
                # shown nondeterministic HW crashes)
                with tc.tile_pool(name="zc", bufs=2) as zpool, \
                     tc.tile_pool(name="ps", bufs=6, space="PSUM") as pspool, \
                     tc.tile_pool(name="tt", bufs=2) as ttpool, \
                     tc.tile_pool(name="gx", bufs=3) as gxpool, \
                     tc.tile_pool(name="gy", bufs=3) as gypool, \
                     tc.tile_pool(name="gi", bufs=6) as gipool, \
                     tc.tile_pool(name="res", bufs=1) as rpool, \
                     tc.tile_pool(name="trp", bufs=2, space="PSUM") as trppool, \
                     tc.tile_pool(name="trs", bufs=2) as trspool:
                    RT = rsize // P  # tiles per range
                    for r in range(nrange):
                      for c0 in range(0, RT, chunk_tiles):
                        ct = min(chunk_tiles, RT - c0)
                        g0 = r * RT + c0  # global tile index
                        zc = zpool.tile([P, chunk_tiles * P], BF16, tag="zc")
                        nc.sync.dma_start(out=zc[:, :ct * P],
                                          in_=z_T[:, g0 * P:(g0 + ct) * P])
                        tt = ttpool.tile([P, chunk_tiles, 2 * H], BF16,
                                         tag="tt")
                        for t in range(ct):
                            ps = pspool.tile([P, 2 * H], F32, tag="ps")
                            nc.tensor.matmul(ps[:],
                                             lhsT=zc[:, t * P:(t + 1) * P],
                                             rhs=wcat_s[:],
                                             start=True, stop=True)
                            # PSUM f32 -> SBUF bf16 (+beta when b1 != 0);
                            # alternate DVE/ACT to balance the engines.
                            if b1_nonzero:
                                nc.vector.scalar_tensor_tensor(
                                    out=tt[:, t, :], in0=ps[:], scalar=1.0,
                                    in1=beta_s[:],
                                    op0=mybir.AluOpType.mult,
                                    op1=mybir.AluOpType.add)
                            elif t % 4 == 0:
                                nc.vector.tensor_copy(tt[:, t, :], ps[:])
                            else:
                                nc.scalar.copy(tt[:, t, :], ps[:])
                        for w0 in range(0, ct, WB_TILES):
                            wt = min(WB_TILES, ct - w0)
                            dst = Ttabs[r][:, c0 + w0:c0 + w0 + wt, :]
                            nc.sync.dma_start(out=dst,
                                              in_=tt[:, w0:w0 + wt, :])

                    # ---- Phase 2: gather + score arcs, 16 (a,b) groups ----
                    resall = rpool.tile([P, rcols_pad], F32, name="resall")
                    nc.vector.memset(resall[:], 0.0)
                    grp_order = sorted(range(ngrp),
                                       key=lambda g: (max(divmod(g, nrange)),
                                                      g))
                    for qi, g in enumerate(grp_order):
                        ga, gb = divmod(g, nrange)
                        ia = gipool.tile([P, S], I16, tag="ia")
                        nc.sync.dma_start(out=ia[:],
                                          in_=isrc[:, g * S:(g + 1) * S])
                        ib = gipool.tile([P, S], I16, tag="ib")
                        nc.sync.dma_start(out=ib[:],
                                          in_=idst[:, g * S:(g + 1) * S])
                        gA = gxpool.tile([P, ctiles, H], BF16, tag="gA")
                        gB = gypool.tile([P, ctiles, H], BF16, tag="gB")
                        # A-half rows of range ga / B-half rows of range gb
                        srcA = Ttabs[ga][:].rearrange(
                            "p t j -> (p t) j")[:, 0:H]
                        srcB = Ttabs[gb][:].rearrange(
                            "p t j -> (p t) j")[:, H:2 * H]
                        # split each gather across queues so several Q7
                        # core-pairs generate descriptors concurrently
                        nsub = max(1, gather_queues)
                        sub_t = ctiles // nsub  # tiles per sub-gather
                        for si in range(nsub):
                            t0 = si * sub_t
                            nt = sub_t if si < nsub - 1 else ctiles - t0
                            n_i = nt * P
                            nc.gpsimd.dma_gather(
                                gA[:, t0:t0 + nt, :], srcA,
                                ia[:, t0 * 8:(t0 + nt) * 8],
                                n_i, n_i, H, elem_step=2 * H,
                                queue_num=(4 * qi + si) % gather_queues,
                                single_packet=False)
                        for si in range(nsub):
                            t0 = si * sub_t
                            nt = sub_t if si < nsub - 1 else ctiles - t0
                            n_i = nt * P
                            nc.gpsimd.dma_gather(
                                gB[:, t0:t0 + nt, :], srcB,
                                ib[:, t0 * 8:(t0 + nt) * 8],
                                n_i, n_i, H, elem_step=2 * H,
                                queue_num=(4 * qi + si + 2) % gather_queues,
                                single_packet=False)
                        if add_via_cce:
                            nc.gpsimd.dma_start(
                                out=gA[:], in_=gB[:],
                                accum_op=mybir.AluOpType.add)
                        else:
                            nc.vector.tensor_tensor(
                                out=gA[:], in0=gA[:], in1=gB[:],
                                op=mybir.AluOpType.add)
                        # fused relu * sign (sign replicated along tiles)
                        sgn_b = sgn_s[:].rearrange(
                            "p (x j) -> p x j", x=1).broadcast_to(
                            [P, ctiles, H])
                        nc.vector.scalar_tensor_tensor(
                            out=gA[:], in0=gA[:], scalar=0.0, in1=sgn_b,
                            op0=mybir.AluOpType.max,
                            op1=mybir.AluOpType.mult)
                        nc.vector.tensor_reduce(
                            out=resall[:, g * ctiles:(g + 1) * ctiles],
                            in_=gA[:], axis=mybir.AxisListType.X,
                            op=mybir.AluOpType.add)

                    # + b2, then transpose 128-col chunks (via PE) and write
                    ident = cpool.tile([P, P], F32, name="ident")
                    make_identity(nc, ident[:])
                    resb = rpool.tile([P, rcols_pad], F32, name="resb")
                    nc.vector.tensor_scalar_add(out=resb[:], in0=resall[:],
                                                scalar1=b2_s[:, 0:1])
                    for m in range(rcols_pad // P):
                        c_lo = m * P
                        c_hi = min(rcols, (m + 1) * P)
                        if c_hi <= c_lo:
                            break
                        w = c_hi - c_lo
                        trp = trppool.tile([P, P], F32, tag="trp")
                        nc.tensor.transpose(out=trp[:],
                                            in_=resb[:, c_lo:c_lo + P],
                                            identity=ident[:])
                        trs = trspool.tile([P, P], F32, tag="trs")
                        nc.vector.tensor_copy(trs[:], trp[:])
                        nc.sync.dma_start(
                            out=outv[c_lo * P:c_hi * P].rearrange(
                                "(c p) -> c p", p=P),
                            in_=trs[:w, :])
    nc.compile()
    return nc


def _host_prep(z, pot_arcs, W1, b1, W2, b2, n_cores=N_CORES):
    """Stage inputs: dtype/layout conversion, arc bucketing, sharding.

    Returns (in_maps, slot) where slot[i] is the device output position of
    arc i within its core's output vector.
    """
    import ml_dtypes

    bf16 = ml_dtypes.bfloat16
    H = HIDDEN
    z = np.asarray(z, np.float32)
    W1 = np.asarray(W1, np.float32)
    b1 = np.asarray(b1, np.float32).reshape(-1)
    W2 = np.asarray(W2, np.float32).reshape(-1)
    b2 = np.asarray(b2, np.float32).reshape(-1)
    arcs = np.asarray(pot_arcs)

    absw2 = np.abs(W2)
    sgn = np.sign(W2).astype(np.float32)
    wsa = (W1[:, :H] * absw2[:, None]).T  # [i, j]
    wsb = (W1[:, H:] * absw2[:, None]).T
    wcat = np.ascontiguousarray(
        np.concatenate([wsa, wsb], axis=1)).astype(bf16)  # [128, 256]
    beta = np.broadcast_to(
        np.concatenate([np.zeros(H, np.float32), absw2 * b1])[None, :],
        (P, 2 * H)).copy().astype(np.float32)
    sgn_rep = np.broadcast_to(sgn[None, :], (P, H)).copy().astype(bf16)
    b2r = np.full((P, 1), b2[0], np.float32)

    zT = np.zeros((P, NODE_PAD), bf16)
    zT[:, :z.shape[0]] = np.ascontiguousarray(z.T).astype(bf16)

    e_per = arcs.shape[0] // n_cores
    S = CAP // 16
    in_maps = []
    slot_all = np.empty(arcs.shape[0], np.int64)
    for c in range(n_cores):
        sh = arcs[c * e_per:(c + 1) * e_per]
        src = np.asarray(sh[:, 0], np.int64)
        dst = np.asarray(sh[:, 1], np.int64)
        grp = (src // RSIZE) * NRANGE + (dst // RSIZE)
        order = np.argsort(grp, kind="stable")
        counts = np.bincount(grp, minlength=NGRP)
        if counts.max() > CAP:
            raise RuntimeError(f"group overflow: {counts.max()} > {CAP}")
        starts = np.zeros(NGRP, np.int64)
        starts[1:] = np.cumsum(counts)[:-1]
        pos_sorted = np.arange(e_per) - starts[grp[order]]
        slot_sorted = grp[order] * CAP + pos_sorted
        slot = np.empty(e_per, np.int64)
        slot[order] = slot_sorted
        slot_all[c * e_per:(c + 1) * e_per] = slot

        la = np.zeros(NGRP * CAP, np.int16)  # padding -> local idx 0 (valid)
        lb = np.zeros(NGRP * CAP, np.int16)
        RT = RSIZE // P
        loca = (src - (src // RSIZE) * RSIZE)[order]
        locb = (dst - (dst // RSIZE) * RSIZE)[order]
        # permuted (partition-major) table row index
        la[slot_sorted] = ((loca % P) * RT + loca // P).astype(np.int16)
        lb[slot_sorted] = ((locb % P) * RT + locb // P).astype(np.int16)
        # wrapped idx layout: position i -> (partition i%16, free i//16),
        # per group; replicated across the 8 Q7 core pairs (128 partitions)
        wa = np.ascontiguousarray(
            la.reshape(NGRP, S, 16).transpose(2, 0, 1).reshape(16, NGRP * S))
        wb = np.ascontiguousarray(
            lb.reshape(NGRP, S, 16).transpose(2, 0, 1).reshape(16, NGRP * S))
        in_maps.append(dict(
            z_T=zT, wcat=wcat, beta=beta, sgn=sgn_rep, b2r=b2r,
            isrc=np.tile(wa, (8, 1)), idst=np.tile(wb, (8, 1))))
    return in_maps, slot_all, e_per


_GRAPH_CACHE = {}


def _get_graph(b1_nonzero):
    key = (NODE_PAD, RSIZE, CAP, CHUNK_TILES, b1_nonzero,
           ADD_VIA_CCE, GATHER_QUEUES)
    if key not in _GRAPH_CACHE:
        _GRAPH_CACHE[key] = _build_graph(NODE_PAD, RSIZE, CAP, CHUNK_TILES,
                                         b1_nonzero=b1_nonzero)
    return _GRAPH_CACHE[key]


def kernel(z, pot_arcs, W1, b1, W2, b2):
    from concourse.bass_utils import run_bass_kernel_spmd

    nc = _get_graph(bool(np.any(np.asarray(b1, np.float32))))
    in_maps, slot, e_per = _host_prep(z, pot_arcs, W1, b1, W2, b2)
    res = run_bass_kernel_spmd(nc, in_maps, core_ids=list(range(N_CORES)))
    out = np.empty(N_ARCS, np.float32)
    for c in range(N_CORES):
        dev = np.asarray(res.results[c]["outv"], np.float32)
        out[c * e_per:(c + 1) * e_per] = dev[slot[c * e_per:(c + 1) * e_per]]
    return out



# revision 25
# speedup vs baseline: 2.5010x; 1.1057x over previous
"""ArcDecoder distributed Bass kernel for 8 TRN2 NeuronCores.

Problem: for each arc e with endpoints (s, d):
    h   = concat(z[s], z[d])                # [256]
    h1  = relu(W1 @ h + b1)                 # [128]
    out = W2 @ h1 + b2                      # scalar

Math transform: W1 @ concat(z_s, z_d) = W1a @ z_s + W1b @ z_d, so per-node
tables are precomputed once (100k nodes instead of 1M arcs):
    A~[n] = (z[n] @ W1a.T) * |W2|,  B~[n] = (z[n] @ W1b.T) * |W2| + |W2|*b1
stored interleaved in bf16 as T[n] = [A~[n], B~[n]] (512B rows).  Then
    out[e] = sum_j sign(W2_j) * relu(A~[s,j] + B~[d,j]) + b2
i.e. per arc: two 256B gathers + an add + one fused max0/mul DVE op + a
segmented reduce.  No per-arc matmul.

Gather: `dma_gather` (the Q7 SWDGE gather) takes int16 indices, so nodes are
split into 4 ranges of 25024 rows and each core's arcs are host-bucketed into
16 (src_range, dst_range) groups; each group's gathers use the range base as
the table offset so all indices fit int16.  Groups are padded to a static
capacity with index-0 dummies; the padding is discarded on the host.

Sharding: arcs split evenly across the 8 cores; z/weights replicated.
No collectives.
"""

import numpy as np

# ---------------- problem constants (hardcoded, per the task spec) ----------
N_NODES = 100000
HIDDEN = 128
N_ARCS = 1000000
N_CORES = 8

P = 128  # SBUF partitions

# ---------------- tiling configuration --------------------------------------
NRANGE = 4
# range size tile-aligned so each range's table is whole 128-node tiles
RSIZE = ((N_NODES + NRANGE * P - 1) // (NRANGE * P)) * P  # 25088
NODE_PAD = NRANGE * RSIZE  # 100352
NGRP = NRANGE * NRANGE  # 16

E_PER_CORE = N_ARCS // N_CORES  # 125000
# group capacity: mean count is E_PER_CORE/16 = 7812.5, sigma ~86 for uniform
# random arcs; 65 tiles = 8320 is ~5.9 sigma above the mean.
CTILES = 65
CAP = CTILES * P  # 8320
E_OUT = NGRP * CAP  # 133120 device outputs per core

CHUNK_TILES = 16  # node tiles per z-chunk DMA in the precompute phase
WB_TILES = 16  # node tiles per T-writeback DMA (1 MB)

# knobs
ADD_VIA_CCE = False  # A += B via SWDGE SBUF->SBUF accumulate DMA (else DVE)
GATHER_QUEUES = 4  # SWDGE queues to spread dma_gathers over (1..4)


def _build_graph(node_pad, rsize, cap, chunk_tiles, b1_nonzero=False,
                 add_via_cce=ADD_VIA_CCE, gather_queues=GATHER_QUEUES):
    """Build the SPMD single-core graph (all 8 cores run this same graph)."""
    import concourse.bass as bass
    from concourse import bacc, mybir, tile
    from concourse.masks import make_identity

    BF16 = mybir.dt.bfloat16
    F32 = mybir.dt.float32
    I16 = mybir.dt.int16
    H = HIDDEN
    NT = node_pad // P
    nrange = node_pad // rsize
    ngrp = nrange * nrange
    ctiles = cap // P
    S = cap // 16  # idx free-dim per group (16-partition wrap)
    e_out = ngrp * cap
    rcols = ngrp * ctiles  # result columns (= e_out / 128)
    rcols_pad = ((rcols + P - 1) // P) * P

    nc = bacc.Bacc(None, target_bir_lowering=False,
                   num_swdge_queues=gather_queues)
    with tile.TileContext(nc) as tc:
        with tc.tile_pool(name="dram", bufs=1, space="DRAM") as dram:
            z_T = dram.tile([P, node_pad], BF16, kind="ExternalInput",
                            name="z_T", uniquify=False)
            wcat = dram.tile([P, 2 * H], BF16, kind="ExternalInput",
                             name="wcat", uniquify=False)
            beta = dram.tile([P, 2 * H], F32, kind="ExternalInput",
                             name="beta", uniquify=False)
            sgn = dram.tile([P, H], BF16, kind="ExternalInput",
                            name="sgn", uniquify=False)
            b2r = dram.tile([P, 1], F32, kind="ExternalInput",
                            name="b2r", uniquify=False)
            isrc = dram.tile([P, ngrp * S], I16, kind="ExternalInput",
                             name="isrc", uniquify=False)
            idst = dram.tile([P, ngrp * S], I16, kind="ExternalInput",
                             name="idst", uniquify=False)
            outv = dram.tile([e_out], F32, kind="ExternalOutput",
                             name="outv", uniquify=False)
            # partition-major table layout: node (local) n lives at
            # [p = n % 128, t = n // 128] so the phase-1 writeback is
            # contiguous 8KB per partition (128 descriptors per chunk
            # instead of 2048 x 512B row descriptors).  Gather row index
            # for node n is (n % 128) * RT + n // 128 (host-computed).
            RT = rsize // P
            Ttabs = [dram.tile([P, RT, 2 * H], BF16, kind="Internal",
                                name=f"Ttab{r}", uniquify=False)
                     for r in range(nrange)]

            with tc.tile_pool(name="consts", bufs=1) as cpool:
                wcat_s = cpool.tile([P, 2 * H], BF16, name="wcat_s")
                nc.sync.dma_start(out=wcat_s[:], in_=wcat[:])
                beta_s = cpool.tile([P, 2 * H], F32, name="beta_s")
                nc.sync.dma_start(out=beta_s[:], in_=beta[:])
                sgn_s = cpool.tile([P, H], BF16, name="sgn_s")
                nc.sync.dma_start(out=sgn_s[:], in_=sgn[:])
                b2_s = cpool.tile([P, 1], F32, name="b2_s")
                nc.sync.dma_start(out=b2_s[:], in_=b2r[:])

                # ---- Phase 1: per-node tables T = [A~ | B~] ----
                # all pools share one scope: phase-2 tiles must NOT alias
                # phase-1 SBUF (aliasing would serialize the phases and has
                # shown nondeterministic HW crashes)
                with tc.tile_pool(name="zc", bufs=2) as zpool, \
                     tc.tile_pool(name="ps", bufs=3, space="PSUM") as pspool, \
                     tc.tile_pool(name="tt", bufs=2) as ttpool, \
                     tc.tile_pool(name="gx", bufs=3) as gxpool, \
                     tc.tile_pool(name="gy", bufs=3) as gypool, \
                     tc.tile_pool(name="gi", bufs=6) as gipool, \
                     tc.tile_pool(name="res", bufs=1) as rpool, \
                     tc.tile_pool(name="trp", bufs=2, space="PSUM") as trppool, \
                     tc.tile_pool(name="trs", bufs=2) as trspool:
                    RT = rsize // P  # tiles per range
                    for r in range(nrange):
                      for c0 in range(0, RT, chunk_tiles):
                        ct = min(chunk_tiles, RT - c0)
                        g0 = r * RT + c0  # global tile index
                        zc = zpool.tile([P, chunk_tiles * P], BF16, tag="zc")
                        nc.sync.dma_start(out=zc[:, :ct * P],
                                          in_=z_T[:, g0 * P:(g0 + ct) * P])
                        tt = ttpool.tile([P, chunk_tiles, 2 * H], BF16,
                                         tag="tt")
                        for t4 in range(0, ct, 4):
                            n4 = min(4, ct - t4)
                            ps = pspool.tile([P, 4, 2 * H], F32, tag="ps")
                            for t in range(t4, t4 + n4):
                                nc.tensor.matmul(ps[:, t - t4, :],
                                                 lhsT=zc[:, t * P:(t + 1) * P],
                                                 rhs=wcat_s[:],
                                                 start=True, stop=True)
                            # batched PSUM f32 -> SBUF bf16 (+beta when
                            # b1 != 0); alternate DVE/ACT per 4-tile block.
                            dst4 = tt[:, t4:t4 + n4, :]
                            src4 = ps[:, :n4, :]
                            if b1_nonzero:
                                beta_b = beta_s[:].rearrange(
                                    "p (x j) -> p x j", x=1).broadcast_to(
                                    [P, n4, 2 * H])
                                nc.vector.scalar_tensor_tensor(
                                    out=dst4, in0=src4, scalar=1.0,
                                    in1=beta_b,
                                    op0=mybir.AluOpType.mult,
                                    op1=mybir.AluOpType.add)
                            elif (t4 // 4) % 4 == 0:
                                nc.vector.tensor_copy(dst4, src4)
                            else:
                                nc.scalar.copy(dst4, src4)
                        for w0 in range(0, ct, WB_TILES):
                            wt = min(WB_TILES, ct - w0)
                            dst = Ttabs[r][:, c0 + w0:c0 + w0 + wt, :]
                            nc.sync.dma_start(out=dst,
                                              in_=tt[:, w0:w0 + wt, :])

                    # ---- Phase 2: gather + score arcs, 16 (a,b) groups ----
                    resall = rpool.tile([P, rcols_pad], F32, name="resall")
                    nc.vector.memset(resall[:], 0.0)
                    grp_order = sorted(range(ngrp),
                                       key=lambda g: (max(divmod(g, nrange)),
                                                      g))
                    for qi, g in enumerate(grp_order):
                        ga, gb = divmod(g, nrange)
                        ia = gipool.tile([P, S], I16, tag="ia")
                        nc.sync.dma_start(out=ia[:],
                                          in_=isrc[:, g * S:(g + 1) * S])
                        ib = gipool.tile([P, S], I16, tag="ib")
                        nc.sync.dma_start(out=ib[:],
                                          in_=idst[:, g * S:(g + 1) * S])
                        gA = gxpool.tile([P, ctiles, H], BF16, tag="gA")
                        gB = gypool.tile([P, ctiles, H], BF16, tag="gB")
                        # A-half rows of range ga / B-half rows of range gb
                        srcA = Ttabs[ga][:].rearrange(
                            "p t j -> (p t) j")[:, 0:H]
                        srcB = Ttabs[gb][:].rearrange(
                            "p t j -> (p t) j")[:, H:2 * H]
                        # split each gather across queues so several Q7
                        # core-pairs generate descriptors concurrently
                        nsub = max(1, gather_queues)
                        sub_t = ctiles // nsub  # tiles per sub-gather
                        for si in range(nsub):
                            t0 = si * sub_t
                            nt = sub_t if si < nsub - 1 else ctiles - t0
                            n_i = nt * P
                            nc.gpsimd.dma_gather(
                                gA[:, t0:t0 + nt, :], srcA,
                                ia[:, t0 * 8:(t0 + nt) * 8],
                                n_i, n_i, H, elem_step=2 * H,
                                queue_num=(4 * qi + si) % gather_queues,
                                single_packet=False)
                        for si in range(nsub):
                            t0 = si * sub_t
                            nt = sub_t if si < nsub - 1 else ctiles - t0
                            n_i = nt * P
                            nc.gpsimd.dma_gather(
                                gB[:, t0:t0 + nt, :], srcB,
                                ib[:, t0 * 8:(t0 + nt) * 8],
                                n_i, n_i, H, elem_step=2 * H,
                                queue_num=(4 * qi + si + 2) % gather_queues,
                                single_packet=False)
                        if add_via_cce:
                            nc.gpsimd.dma_start(
                                out=gA[:], in_=gB[:],
                                accum_op=mybir.AluOpType.add)
                        else:
                            nc.vector.tensor_tensor(
                                out=gA[:], in0=gA[:], in1=gB[:],
                                op=mybir.AluOpType.add)
                        # fused relu * sign (sign replicated along tiles)
                        sgn_b = sgn_s[:].rearrange(
                            "p (x j) -> p x j", x=1).broadcast_to(
                            [P, ctiles, H])
                        nc.vector.scalar_tensor_tensor(
                            out=gA[:], in0=gA[:], scalar=0.0, in1=sgn_b,
                            op0=mybir.AluOpType.max,
                            op1=mybir.AluOpType.mult)
                        nc.vector.tensor_reduce(
                            out=resall[:, g * ctiles:(g + 1) * ctiles],
                            in_=gA[:], axis=mybir.AxisListType.X,
                            op=mybir.AluOpType.add)

                    # + b2, then transpose 128-col chunks (via PE) and write
                    ident = cpool.tile([P, P], F32, name="ident")
                    make_identity(nc, ident[:])
                    resb = rpool.tile([P, rcols_pad], F32, name="resb")
                    nc.vector.tensor_scalar_add(out=resb[:], in0=resall[:],
                                                scalar1=b2_s[:, 0:1])
                    for m in range(rcols_pad // P):
                        c_lo = m * P
                        c_hi = min(rcols, (m + 1) * P)
                        if c_hi <= c_lo:
                            break
                        w = c_hi - c_lo
                        trp = trppool.tile([P, P], F32, tag="trp")
                        nc.tensor.transpose(out=trp[:],
                                            in_=resb[:, c_lo:c_lo + P],
                                            identity=ident[:])
                        trs = trspool.tile([P, P], F32, tag="trs")
                        nc.vector.tensor_copy(trs[:], trp[:])
                        nc.sync.dma_start(
                            out=outv[c_lo * P:c_hi * P].rearrange(
                                "(c p) -> c p", p=P),
                            in_=trs[:w, :])
    nc.compile()
    return nc


def _host_prep(z, pot_arcs, W1, b1, W2, b2, n_cores=N_CORES):
    """Stage inputs: dtype/layout conversion, arc bucketing, sharding.

    Returns (in_maps, slot) where slot[i] is the device output position of
    arc i within its core's output vector.
    """
    import ml_dtypes

    bf16 = ml_dtypes.bfloat16
    H = HIDDEN
    z = np.asarray(z, np.float32)
    W1 = np.asarray(W1, np.float32)
    b1 = np.asarray(b1, np.float32).reshape(-1)
    W2 = np.asarray(W2, np.float32).reshape(-1)
    b2 = np.asarray(b2, np.float32).reshape(-1)
    arcs = np.asarray(pot_arcs)

    absw2 = np.abs(W2)
    sgn = np.sign(W2).astype(np.float32)
    wsa = (W1[:, :H] * absw2[:, None]).T  # [i, j]
    wsb = (W1[:, H:] * absw2[:, None]).T
    wcat = np.ascontiguousarray(
        np.concatenate([wsa, wsb], axis=1)).astype(bf16)  # [128, 256]
    beta = np.broadcast_to(
        np.concatenate([np.zeros(H, np.float32), absw2 * b1])[None, :],
        (P, 2 * H)).copy().astype(np.float32)
    sgn_rep = np.broadcast_to(sgn[None, :], (P, H)).copy().astype(bf16)
    b2r = np.full((P, 1), b2[0], np.float32)

    zT = np.zeros((P, NODE_PAD), bf16)
    zT[:, :z.shape[0]] = np.ascontiguousarray(z.T).astype(bf16)

    e_per = arcs.shape[0] // n_cores
    S = CAP // 16
    in_maps = []
    slot_all = np.empty(arcs.shape[0], np.int64)
    for c in range(n_cores):
        sh = arcs[c * e_per:(c + 1) * e_per]
        src = np.asarray(sh[:, 0], np.int64)
        dst = np.asarray(sh[:, 1], np.int64)
        grp = (src // RSIZE) * NRANGE + (dst // RSIZE)
        order = np.argsort(grp, kind="stable")
        counts = np.bincount(grp, minlength=NGRP)
        if counts.max() > CAP:
            raise RuntimeError(f"group overflow: {counts.max()} > {CAP}")
        starts = np.zeros(NGRP, np.int64)
        starts[1:] = np.cumsum(counts)[:-1]
        pos_sorted = np.arange(e_per) - starts[grp[order]]
        slot_sorted = grp[order] * CAP + pos_sorted
        slot = np.empty(e_per, np.int64)
        slot[order] = slot_sorted
        slot_all[c * e_per:(c + 1) * e_per] = slot

        la = np.zeros(NGRP * CAP, np.int16)  # padding -> local idx 0 (valid)
        lb = np.zeros(NGRP * CAP, np.int16)
        RT = RSIZE // P
        loca = (src - (src // RSIZE) * RSIZE)[order]
        locb = (dst - (dst // RSIZE) * RSIZE)[order]
        # permuted (partition-major) table row index
        la[slot_sorted] = ((loca % P) * RT + loca // P).astype(np.int16)
        lb[slot_sorted] = ((locb % P) * RT + locb // P).astype(np.int16)
        # wrapped idx layout: position i -> (partition i%16, free i//16),
        # per group; replicated across the 8 Q7 core pairs (128 partitions)
        wa = np.ascontiguousarray(
            la.reshape(NGRP, S, 16).transpose(2, 0, 1).reshape(16, NGRP * S))
        wb = np.ascontiguousarray(
            lb.reshape(NGRP, S, 16).transpose(2, 0, 1).reshape(16, NGRP * S))
        in_maps.append(dict(
            z_T=zT, wcat=wcat, beta=beta, sgn=sgn_rep, b2r=b2r,
            isrc=np.tile(wa, (8, 1)), idst=np.tile(wb, (8, 1))))
    return in_maps, slot_all, e_per


_GRAPH_CACHE = {}


def _get_graph(b1_nonzero):
    key = (NODE_PAD, RSIZE, CAP, CHUNK_TILES, b1_nonzero,
           ADD_VIA_CCE, GATHER_QUEUES)
    if key not in _GRAPH_CACHE:
        _GRAPH_CACHE[key] = _build_graph(NODE_PAD, RSIZE, CAP, CHUNK_TILES,
                                         b1_nonzero=b1_nonzero)
    return _GRAPH_CACHE[key]


def kernel(z, pot_arcs, W1, b1, W2, b2):
    from concourse.bass_utils import run_bass_kernel_spmd

    nc = _get_graph(bool(np.any(np.asarray(b1, np.float32))))
    in_maps, slot, e_per = _host_prep(z, pot_arcs, W1, b1, W2, b2)
    res = run_bass_kernel_spmd(nc, in_maps, core_ids=list(range(N_CORES)))
    out = np.empty(N_ARCS, np.float32)
    for c in range(N_CORES):
        dev = np.asarray(res.results[c]["outv"], np.float32)
        out[c * e_per:(c + 1) * e_per] = dev[slot[c * e_per:(c + 1) * e_per]]
    return out



# revision 26
# speedup vs baseline: 2.5527x; 1.0207x over previous
"""ArcDecoder distributed Bass kernel for 8 TRN2 NeuronCores.

Problem: for each arc e with endpoints (s, d):
    h   = concat(z[s], z[d])                # [256]
    h1  = relu(W1 @ h + b1)                 # [128]
    out = W2 @ h1 + b2                      # scalar

Math transform: W1 @ concat(z_s, z_d) = W1a @ z_s + W1b @ z_d, so per-node
tables are precomputed once (100k nodes instead of 1M arcs):
    A~[n] = (z[n] @ W1a.T) * |W2|,  B~[n] = (z[n] @ W1b.T) * |W2| + |W2|*b1
stored interleaved in bf16 as T[n] = [A~[n], B~[n]] (512B rows).  Then
    out[e] = sum_j sign(W2_j) * relu(A~[s,j] + B~[d,j]) + b2
i.e. per arc: two 256B gathers + an add + one fused max0/mul DVE op + a
segmented reduce.  No per-arc matmul.

Gather: `dma_gather` (the Q7 SWDGE gather) takes int16 indices, so nodes are
split into 4 ranges of 25024 rows and each core's arcs are host-bucketed into
16 (src_range, dst_range) groups; each group's gathers use the range base as
the table offset so all indices fit int16.  Groups are padded to a static
capacity with index-0 dummies; the padding is discarded on the host.

Sharding: arcs split evenly across the 8 cores; z/weights replicated.
No collectives.
"""

import numpy as np

# ---------------- problem constants (hardcoded, per the task spec) ----------
N_NODES = 100000
HIDDEN = 128
N_ARCS = 1000000
N_CORES = 8

P = 128  # SBUF partitions

# ---------------- tiling configuration --------------------------------------
NRANGE = 4
# range size tile-aligned so each range's table is whole 128-node tiles
RSIZE = ((N_NODES + NRANGE * P - 1) // (NRANGE * P)) * P  # 25088
NODE_PAD = NRANGE * RSIZE  # 100352
NGRP = NRANGE * NRANGE  # 16

E_PER_CORE = N_ARCS // N_CORES  # 125000
# group capacity: mean count is E_PER_CORE/16 = 7812.5, sigma ~86 for uniform
# random arcs; 65 tiles = 8320 is ~5.9 sigma above the mean.
CTILES = 65
CAP = CTILES * P  # 8320
E_OUT = NGRP * CAP  # 133120 device outputs per core

CHUNK_TILES = 16  # node tiles per z-chunk DMA in the precompute phase
WB_TILES = 16  # node tiles per T-writeback DMA (1 MB)

# knobs
ADD_VIA_CCE = False  # A += B via SWDGE SBUF->SBUF accumulate DMA (else DVE)
GATHER_QUEUES = 4  # SWDGE queues to spread dma_gathers over (1..4)


def _build_graph(node_pad, rsize, cap, chunk_tiles, b1_nonzero=False,
                 add_via_cce=ADD_VIA_CCE, gather_queues=GATHER_QUEUES):
    """Build the SPMD single-core graph (all 8 cores run this same graph)."""
    import concourse.bass as bass
    from concourse import bacc, mybir, tile
    from concourse.masks import make_identity

    BF16 = mybir.dt.bfloat16
    F32 = mybir.dt.float32
    I16 = mybir.dt.int16
    H = HIDDEN
    NT = node_pad // P
    nrange = node_pad // rsize
    ngrp = nrange * nrange
    ctiles = cap // P
    S = cap // 16  # idx free-dim per group (16-partition wrap)
    e_out = ngrp * cap
    rcols = ngrp * ctiles  # result columns (= e_out / 128)
    rcols_pad = ((rcols + P - 1) // P) * P

    nc = bacc.Bacc(None, target_bir_lowering=False,
                   num_swdge_queues=gather_queues)
    with tile.TileContext(nc) as tc:
        with tc.tile_pool(name="dram", bufs=1, space="DRAM") as dram:
            z_T = dram.tile([P, node_pad], BF16, kind="ExternalInput",
                            name="z_T", uniquify=False)
            wcat = dram.tile([P, 2 * H], BF16, kind="ExternalInput",
                             name="wcat", uniquify=False)
            beta = dram.tile([P, 2 * H], F32, kind="ExternalInput",
                             name="beta", uniquify=False)
            sgn = dram.tile([P, H], BF16, kind="ExternalInput",
                            name="sgn", uniquify=False)
            b2r = dram.tile([P, 1], F32, kind="ExternalInput",
                            name="b2r", uniquify=False)
            isrc = dram.tile([P, ngrp * S], I16, kind="ExternalInput",
                             name="isrc", uniquify=False)
            idst = dram.tile([P, ngrp * S], I16, kind="ExternalInput",
                             name="idst", uniquify=False)
            outv = dram.tile([e_out], F32, kind="ExternalOutput",
                             name="outv", uniquify=False)
            # partition-major table layout: node (local) n lives at
            # [p = n % 128, t = n // 128] so the phase-1 writeback is
            # contiguous 8KB per partition (128 descriptors per chunk
            # instead of 2048 x 512B row descriptors).  Gather row index
            # for node n is (n % 128) * RT + n // 128 (host-computed).
            RT = rsize // P
            Ttabs = [dram.tile([P, RT, 2 * H], BF16, kind="Internal",
                                name=f"Ttab{r}", uniquify=False)
                     for r in range(nrange)]

            with tc.tile_pool(name="consts", bufs=1) as cpool:
                wcat_s = cpool.tile([P, 2 * H], BF16, name="wcat_s")
                nc.sync.dma_start(out=wcat_s[:], in_=wcat[:])
                beta_s = cpool.tile([P, 2 * H], F32, name="beta_s")
                nc.sync.dma_start(out=beta_s[:], in_=beta[:])
                sgn_s = cpool.tile([P, H], BF16, name="sgn_s")
                nc.sync.dma_start(out=sgn_s[:], in_=sgn[:])
                b2_s = cpool.tile([P, 1], F32, name="b2_s")
                nc.sync.dma_start(out=b2_s[:], in_=b2r[:])

                # ---- Phase 1: per-node tables T = [A~ | B~] ----
                # all pools share one scope: phase-2 tiles must NOT alias
                # phase-1 SBUF (aliasing would serialize the phases and has
                # shown nondeterministic HW crashes)
                with tc.tile_pool(name="zc", bufs=2) as zpool, \
                     tc.tile_pool(name="ps", bufs=3, space="PSUM") as pspool, \
                     tc.tile_pool(name="tt", bufs=2) as ttpool, \
                     tc.tile_pool(name="gx", bufs=3) as gxpool, \
                     tc.tile_pool(name="gy", bufs=3) as gypool, \
                     tc.tile_pool(name="gi", bufs=6) as gipool, \
                     tc.tile_pool(name="res", bufs=1) as rpool, \
                     tc.tile_pool(name="trp", bufs=2, space="PSUM") as trppool, \
                     tc.tile_pool(name="trs", bufs=2) as trspool:
                    RT = rsize // P  # tiles per range
                    for r in range(nrange):
                      for c0 in range(0, RT, chunk_tiles):
                        ct = min(chunk_tiles, RT - c0)
                        g0 = r * RT + c0  # global tile index
                        zc = zpool.tile([P, chunk_tiles * P], BF16, tag="zc")
                        nc.sync.dma_start(out=zc[:, :ct * P],
                                          in_=z_T[:, g0 * P:(g0 + ct) * P])
                        tt = ttpool.tile([P, chunk_tiles, 2 * H], BF16,
                                         tag="tt")
                        for t4 in range(0, ct, 4):
                            n4 = min(4, ct - t4)
                            ps = pspool.tile([P, 4, 2 * H], F32, tag="ps")
                            for t in range(t4, t4 + n4):
                                nc.tensor.matmul(ps[:, t - t4, :],
                                                 lhsT=zc[:, t * P:(t + 1) * P],
                                                 rhs=wcat_s[:],
                                                 start=True, stop=True)
                            # batched PSUM f32 -> SBUF bf16 (+beta when
                            # b1 != 0); alternate DVE/ACT per 4-tile block.
                            dst4 = tt[:, t4:t4 + n4, :]
                            src4 = ps[:, :n4, :]
                            if b1_nonzero:
                                beta_b = beta_s[:].rearrange(
                                    "p (x j) -> p x j", x=1).broadcast_to(
                                    [P, n4, 2 * H])
                                nc.vector.scalar_tensor_tensor(
                                    out=dst4, in0=src4, scalar=1.0,
                                    in1=beta_b,
                                    op0=mybir.AluOpType.mult,
                                    op1=mybir.AluOpType.add)
                            elif (t4 // 4) % 4 == 0:
                                nc.vector.tensor_copy(dst4, src4)
                            else:
                                nc.scalar.copy(dst4, src4)
                        for w0 in range(0, ct, WB_TILES):
                            wt = min(WB_TILES, ct - w0)
                            dst = Ttabs[r][:, c0 + w0:c0 + w0 + wt, :]
                            nc.sync.dma_start(out=dst,
                                              in_=tt[:, w0:w0 + wt, :])

                    # ---- Phase 2: gather + score arcs, 16 (a,b) groups ----
                    resall = rpool.tile([P, rcols_pad], F32, name="resall")
                    nc.vector.memset(resall[:], 0.0)
                    grp_order = sorted(range(ngrp),
                                       key=lambda g: (max(divmod(g, nrange)),
                                                      g))
                    for qi, g in enumerate(grp_order):
                        ga, gb = divmod(g, nrange)
                        ia = gipool.tile([P, S], I16, tag="ia")
                        nc.sync.dma_start(out=ia[:],
                                          in_=isrc[:, g * S:(g + 1) * S])
                        ib = gipool.tile([P, S], I16, tag="ib")
                        nc.sync.dma_start(out=ib[:],
                                          in_=idst[:, g * S:(g + 1) * S])
                        gA = gxpool.tile([P, ctiles, H], BF16, tag="gA")
                        gB = gypool.tile([P, ctiles, H], BF16, tag="gB")
                        # A-half rows of range ga / B-half rows of range gb
                        srcA = Ttabs[ga][:].rearrange(
                            "p t j -> (p t) j")[:, 0:H]
                        srcB = Ttabs[gb][:].rearrange(
                            "p t j -> (p t) j")[:, H:2 * H]
                        # split each gather across queues so several Q7
                        # core-pairs generate descriptors concurrently.
                        # Queue-0 generation runs INLINE on the Pool engine
                        # (blocks dispatch), queues 1-3 run async on other
                        # Q7 pairs -- so emit the async ops FIRST and give
                        # queue 0 the last sub of each direction.
                        sub_t = ctiles // 4  # tiles per sub-gather
                        subs = [(si * sub_t,
                                 sub_t if si < 3 else ctiles - 3 * sub_t)
                                for si in range(4)]
                        plan = [("A", 0, 1), ("A", 1, 2), ("B", 0, 3),
                                ("B", 1, 1), ("A", 2, 2), ("B", 2, 3),
                                ("A", 3, 0), ("B", 3, 0)]
                        for d, si, q in plan:
                            t0, nt = subs[si]
                            n_i = nt * P
                            buf, src, idx = ((gA, srcA, ia) if d == "A"
                                             else (gB, srcB, ib))
                            nc.gpsimd.dma_gather(
                                buf[:, t0:t0 + nt, :], src,
                                idx[:, t0 * 8:(t0 + nt) * 8],
                                n_i, n_i, H, elem_step=2 * H,
                                queue_num=q % gather_queues,
                                single_packet=False)
                        # add + relu*sgn + reduce in two halves so the DVE
                        # chain starts before the whole group is gathered
                        for h0, ht in ((0, 2 * sub_t),
                                       (2 * sub_t, ctiles - 2 * sub_t)):
                            ga_h = gA[:, h0:h0 + ht, :]
                            nc.vector.tensor_tensor(
                                out=ga_h, in0=ga_h,
                                in1=gB[:, h0:h0 + ht, :],
                                op=mybir.AluOpType.add)
                            sgn_b = sgn_s[:].rearrange(
                                "p (x j) -> p x j", x=1).broadcast_to(
                                [P, ht, H])
                            nc.vector.scalar_tensor_tensor(
                                out=ga_h, in0=ga_h, scalar=0.0, in1=sgn_b,
                                op0=mybir.AluOpType.max,
                                op1=mybir.AluOpType.mult)
                            nc.vector.tensor_reduce(
                                out=resall[:, g * ctiles + h0:
                                           g * ctiles + h0 + ht],
                                in_=ga_h, axis=mybir.AxisListType.X,
                                op=mybir.AluOpType.add)

                    # + b2, then transpose 128-col chunks (via PE) and write
                    ident = cpool.tile([P, P], F32, name="ident")
                    make_identity(nc, ident[:])
                    resb = rpool.tile([P, rcols_pad], F32, name="resb")
                    nc.vector.tensor_scalar_add(out=resb[:], in0=resall[:],
                                                scalar1=b2_s[:, 0:1])
                    for m in range(rcols_pad // P):
                        c_lo = m * P
                        c_hi = min(rcols, (m + 1) * P)
                        if c_hi <= c_lo:
                            break
                        w = c_hi - c_lo
                        trp = trppool.tile([P, P], F32, tag="trp")
                        nc.tensor.transpose(out=trp[:],
                                            in_=resb[:, c_lo:c_lo + P],
                                            identity=ident[:])
                        trs = trspool.tile([P, P], F32, tag="trs")
                        nc.vector.tensor_copy(trs[:], trp[:])
                        nc.sync.dma_start(
                            out=outv[c_lo * P:c_hi * P].rearrange(
                                "(c p) -> c p", p=P),
                            in_=trs[:w, :])
    nc.compile()
    return nc


def _host_prep(z, pot_arcs, W1, b1, W2, b2, n_cores=N_CORES):
    """Stage inputs: dtype/layout conversion, arc bucketing, sharding.

    Returns (in_maps, slot) where slot[i] is the device output position of
    arc i within its core's output vector.
    """
    import ml_dtypes

    bf16 = ml_dtypes.bfloat16
    H = HIDDEN
    z = np.asarray(z, np.float32)
    W1 = np.asarray(W1, np.float32)
    b1 = np.asarray(b1, np.float32).reshape(-1)
    W2 = np.asarray(W2, np.float32).reshape(-1)
    b2 = np.asarray(b2, np.float32).reshape(-1)
    arcs = np.asarray(pot_arcs)

    absw2 = np.abs(W2)
    sgn = np.sign(W2).astype(np.float32)
    wsa = (W1[:, :H] * absw2[:, None]).T  # [i, j]
    wsb = (W1[:, H:] * absw2[:, None]).T
    wcat = np.ascontiguousarray(
        np.concatenate([wsa, wsb], axis=1)).astype(bf16)  # [128, 256]
    beta = np.broadcast_to(
        np.concatenate([np.zeros(H, np.float32), absw2 * b1])[None, :],
        (P, 2 * H)).copy().astype(np.float32)
    sgn_rep = np.broadcast_to(sgn[None, :], (P, H)).copy().astype(bf16)
    b2r = np.full((P, 1), b2[0], np.float32)

    zT = np.zeros((P, NODE_PAD), bf16)
    zT[:, :z.shape[0]] = np.ascontiguousarray(z.T).astype(bf16)

    e_per = arcs.shape[0] // n_cores
    S = CAP // 16
    in_maps = []
    slot_all = np.empty(arcs.shape[0], np.int64)
    for c in range(n_cores):
        sh = arcs[c * e_per:(c + 1) * e_per]
        src = np.asarray(sh[:, 0], np.int64)
        dst = np.asarray(sh[:, 1], np.int64)
        grp = (src // RSIZE) * NRANGE + (dst // RSIZE)
        order = np.argsort(grp, kind="stable")
        counts = np.bincount(grp, minlength=NGRP)
        if counts.max() > CAP:
            raise RuntimeError(f"group overflow: {counts.max()} > {CAP}")
        starts = np.zeros(NGRP, np.int64)
        starts[1:] = np.cumsum(counts)[:-1]
        pos_sorted = np.arange(e_per) - starts[grp[order]]
        slot_sorted = grp[order] * CAP + pos_sorted
        slot = np.empty(e_per, np.int64)
        slot[order] = slot_sorted
        slot_all[c * e_per:(c + 1) * e_per] = slot

        la = np.zeros(NGRP * CAP, np.int16)  # padding -> local idx 0 (valid)
        lb = np.zeros(NGRP * CAP, np.int16)
        RT = RSIZE // P
        loca = (src - (src // RSIZE) * RSIZE)[order]
        locb = (dst - (dst // RSIZE) * RSIZE)[order]
        # permuted (partition-major) table row index
        la[slot_sorted] = ((loca % P) * RT + loca // P).astype(np.int16)
        lb[slot_sorted] = ((locb % P) * RT + locb // P).astype(np.int16)
        # wrapped idx layout: position i -> (partition i%16, free i//16),
        # per group; replicated across the 8 Q7 core pairs (128 partitions)
        wa = np.ascontiguousarray(
            la.reshape(NGRP, S, 16).transpose(2, 0, 1).reshape(16, NGRP * S))
        wb = np.ascontiguousarray(
            lb.reshape(NGRP, S, 16).transpose(2, 0, 1).reshape(16, NGRP * S))
        in_maps.append(dict(
            z_T=zT, wcat=wcat, beta=beta, sgn=sgn_rep, b2r=b2r,
            isrc=np.tile(wa, (8, 1)), idst=np.tile(wb, (8, 1))))
    return in_maps, slot_all, e_per


_GRAPH_CACHE = {}


def _get_graph(b1_nonzero):
    key = (NODE_PAD, RSIZE, CAP, CHUNK_TILES, b1_nonzero,
           ADD_VIA_CCE, GATHER_QUEUES)
    if key not in _GRAPH_CACHE:
        _GRAPH_CACHE[key] = _build_graph(NODE_PAD, RSIZE, CAP, CHUNK_TILES,
                                         b1_nonzero=b1_nonzero)
    return _GRAPH_CACHE[key]


def kernel(z, pot_arcs, W1, b1, W2, b2):
    from concourse.bass_utils import run_bass_kernel_spmd

    nc = _get_graph(bool(np.any(np.asarray(b1, np.float32))))
    in_maps, slot, e_per = _host_prep(z, pot_arcs, W1, b1, W2, b2)
    res = run_bass_kernel_spmd(nc, in_maps, core_ids=list(range(N_CORES)))
    out = np.empty(N_ARCS, np.float32)
    for c in range(N_CORES):
        dev = np.asarray(res.results[c]["outv"], np.float32)
        out[c * e_per:(c + 1) * e_per] = dev[slot[c * e_per:(c + 1) * e_per]]
    return out

